# revision 2
# baseline (speedup 1.0000x reference)
"""KronEmbedding lookup kernel for 8 TRN2 NeuronCores.

Math: w = einsum('sia,sjb->ijab', A, B).reshape(50176, 2048); out = w[x].
Never materializes w. Per token t with i=x//224, j=x%224:
    out[t] = sum_s outer(A[s,i,:], B[s,j,:])   -> (64*32 = 2048 floats)

Strategy (data-parallel over tokens, 1024 tokens/core):
- Host: repack A -> A3[8i+s, a] (rows 256B), B -> B3[8j+s, b-padded-to-64],
  and build per-(token,s) gather indices in the SWDGE wrapped-int16 layout.
- Device per 128-token tile:
    dma_gather A-rows -> stacked lhsT layout [(8k+s)%128, group, 64]
    dma_gather B-rows -> same layout
    round fp32 -> fp32r (DVE copy) for full-rate PE matmuls
    16 strided SBUF->SBUF DMAs scatter B rows into a block-diagonal moving
      operand BD[(8k+s), (k,b)] (off-diag zeros persist across tiles)
    per 16-token group: matmul(out[a, (k,b)] = Ag_stacked^T @ BD), two
      groups packed per PSUM tile on partition halves
    evacuate PSUM -> SBUF (DVE/ACT alternating), DMA 256KB blocks to HBM
- Host: reorder device-native [tile, pair, 128, 512] blocks to token-major.
"""
import numpy as np
from contextlib import ExitStack

import concourse.bass as bass
import concourse.bacc as bacc
import concourse.tile as tile
import concourse.mybir as mybir
from concourse import bass_utils

dt = mybir.dt

R, M1, N1, M2, N2 = 8, 224, 64, 224, 32
VOCAB, EMB = M1 * M2, N1 * N2          # 50176, 2048
BATCH, SEQ = 4, 2048
NTOK = BATCH * SEQ                     # 8192
NCORES = 8
TPC = NTOK // NCORES                   # 1024 tokens per core
NTILES = TPC // 128                    # 8 tiles of 128 tokens
NGRP = 8                               # 16-token groups per tile

_CACHE = {}


def _build():
    nc = bacc.Bacc("TRN2", num_devices=NCORES)
    A3 = nc.dram_tensor("A3", [M1 * R, 64], dt.float32, kind="ExternalInput")
    B3 = nc.dram_tensor("B3", [M2 * R, 64], dt.float32, kind="ExternalInput")
    idxA = nc.dram_tensor("idxA", [128, TPC * 8 // 16], dt.int16, kind="ExternalInput")
    idxB = nc.dram_tensor("idxB", [128, TPC * 8 // 16], dt.int16, kind="ExternalInput")
    out = nc.dram_tensor("out", [NTILES, 4, 64, 1024], dt.float32, kind="ExternalOutput")

    with tile.TileContext(nc) as tc, ExitStack() as ctx:
        const_pool = ctx.enter_context(tc.tile_pool(name="const", bufs=1))
        agf_pool = ctx.enter_context(tc.tile_pool(name="agf", bufs=3))
        bgf_pool = ctx.enter_context(tc.tile_pool(name="bgf", bufs=3))
        agr_pool = ctx.enter_context(tc.tile_pool(name="agr", bufs=3))
        bgr_pool = ctx.enter_context(tc.tile_pool(name="bgr", bufs=3))
        ev_pool = ctx.enter_context(tc.tile_pool(name="ev", bufs=6))
        ps_pool = ctx.enter_context(tc.tile_pool(name="ps", bufs=3, space="PSUM"))

        idxA_sb = const_pool.tile([128, 512], dt.int16, tag="idxA")
        idxB_sb = const_pool.tile([128, 512], dt.int16, tag="idxB")
        nc.sync.dma_start(idxA_sb[:], idxA[:])
        nc.sync.dma_start(idxB_sb[:], idxB[:])

        # Two persistent block-diagonal buffers (double buffer by hand so the
        # off-diagonal zeros are written exactly once).
        bd_bufs = [
            const_pool.tile([128, NGRP, 512], dt.float32r, tag=f"bd{i}", name=f"bd{i}")
            for i in range(2)
        ]
        for b in bd_bufs:
            nc.gpsimd.memset(b[:].bitcast(dt.float32), 0.0)

        for t in range(NTILES):
            agf = agf_pool.tile([128, NGRP, 64], dt.float32, tag="agf")
            nc.gpsimd.dma_gather(
                agf[:], A3[:], idxA_sb[:, 64 * t:64 * (t + 1)], 1024, 1024, 64
            )
            bgf = bgf_pool.tile([128, NGRP, 64], dt.float32, tag="bgf")
            nc.gpsimd.dma_gather(
                bgf[:], B3[:], idxB_sb[:, 64 * t:64 * (t + 1)], 1024, 1024, 64
            )
            agr = agr_pool.tile([128, NGRP, 64], dt.float32r, tag="agr")
            nc.vector.tensor_copy(agr[:], agf[:])
            bgr = bgr_pool.tile([128, NGRP, 64], dt.float32r, tag="bgr")
            nc.vector.tensor_copy(bgr[:], bgf[:])

            bd = bd_bufs[t % 2]
            for k in range(16):
                nc.sync.dma_start(
                    bd[8 * k:8 * k + 8, :, 32 * k:32 * k + 32],
                    bgr[8 * k:8 * k + 8, :, 0:32],
                )

            for pair in range(4):
                ps = ps_pool.tile([64, 1024], dt.float32, tag="ps")
                for h in range(2):
                    g = 2 * pair + h
                    nc.tensor.matmul(
                        ps[:, 512 * h:512 * h + 512],
                        agr[:, g, :],
                        bd[:, g, :],
                        start=True,
                        stop=True,
                    )
                ev = ev_pool.tile([64, 1024], dt.float32, tag="ev")
                if pair % 2 == 0:
                    nc.vector.tensor_copy(ev[:], ps[:])
                else:
                    nc.scalar.copy(ev[:], ps[:])
                nc.sync.dma_start(out[t, pair], ev[:])

    nc.compile()
    return nc


def _wrap_idxs(idx: np.ndarray) -> np.ndarray:
    """[n] -> SWDGE wrapped layout [128, n//16] int16 (16-wrap, 8x replicated)."""
    n = idx.shape[0]
    w = idx.reshape(n // 16, 16).T.astype(np.int16)
    return np.ascontiguousarray(np.tile(w, (8, 1)))


def kernel(A: np.ndarray, B: np.ndarray, x: np.ndarray) -> np.ndarray:
    A = np.asarray(A, dtype=np.float32)
    B = np.asarray(B, dtype=np.float32)
    xl = np.asarray(x).astype(np.int64).reshape(-1)           # [8192]

    A3 = np.ascontiguousarray(A.transpose(1, 0, 2).reshape(M1 * R, 64))
    B3 = np.zeros((M2 * R, 64), dtype=np.float32)
    B3[:, :32] = B.transpose(1, 0, 2).reshape(M2 * R, 32)

    i_all = (xl // M2).astype(np.int64)
    j_all = (xl % M2).astype(np.int64)

    if "nc" not in _CACHE:
        _CACHE["nc"] = _build()
    nc = _CACHE["nc"]

    s = np.arange(R, dtype=np.int64)
    in_maps = []
    for c in range(NCORES):
        sl = slice(c * TPC, (c + 1) * TPC)
        ia = (i_all[sl, None] * R + s[None, :]).reshape(-1)   # [8192] per core
        jb = (j_all[sl, None] * R + s[None, :]).reshape(-1)
        in_maps.append(
            dict(A3=A3, B3=B3, idxA=_wrap_idxs(ia), idxB=_wrap_idxs(jb))
        )

    _CACHE["in_maps"] = in_maps
    res = bass_utils.run_bass_kernel_spmd(nc, in_maps, core_ids=list(range(NCORES)))

    outs = []
    for c in range(NCORES):
        o = res.results[c]["out"]                      # [8, 4, 128, 512]
        o = o.reshape(NTILES, 4, 64, 2, 16, 32)        # [t, p, a, gh, k, b]
        o = o.transpose(0, 1, 3, 4, 2, 5)              # [t, p, gh, k, a, b]
        outs.append(o.reshape(TPC, EMB))
    full = np.concatenate(outs, axis=0)                # [8192, 2048]
    return full.reshape(BATCH, SEQ, EMB)



# revision 3
# speedup vs baseline: 1.0551x; 1.0551x over previous
"""KronEmbedding lookup kernel v4 — s-pair-packed SWDGE gathers.

Math: w = einsum('sia,sjb->ijab', A, B).reshape(50176, 2048); out = w[x].
Per token T (i = x//224, j = x%224):
    out[T] = sum_s outer(A[s,i,:], B[s,j,:])

SWDGE descriptor emission costs ~8.4ns/desc serial on GpSimd; the v1/v2
layout needs 8 descs/token (one per s) = 16K descs = ~140us. Here each
256B descriptor carries an s-PAIR row, so 4 descs/token = 8K descs:
- A3p row (4i+p) = [A[2p,i,:] | A[2p+1,i,:]]            (128 bf16 els)
- B3p row (4j+p) = [B[2p,j,:] pad32 | B[2p+1,j,:] pad32] (128 els)
- gather position q = token*4 + p -> partition 4*(token%32)+p, col
  c = token//32 (32-token blocks; c = 4*tile + cb).
The s-pair parity sigma lives in the FREE dim, so each output block is
computed as TWO PSUM-accumulated matmuls (sigma = 0, 1), with K = 128
partitions = 32 tokens x 4 pairs:
  psum[a, (k'',b)] += Agp[:, c, 64s:64s+64]^T @ bd[:, ..., sigma, h, :]
The block-diagonal moving operand covers 4 tiles at a time (SBUF), built
by 32 per-k' DMAs per chunk; diagonal cells are rewritten every chunk so
zeros are memset once.
"""
import numpy as np
from contextlib import ExitStack

import ml_dtypes

import concourse.bass as bass
import concourse.bacc as bacc
import concourse.tile as tile
import concourse.mybir as mybir
from concourse import bass_utils

dt = mybir.dt

R, M1, N1, M2, N2 = 8, 224, 64, 224, 32
VOCAB, EMB = M1 * M2, N1 * N2          # 50176, 2048
BATCH, SEQ = 4, 2048
NTOK = BATCH * SEQ                     # 8192
NCORES = 8
TPC = NTOK // NCORES                   # 1024 tokens per core
NTILES = TPC // 128                    # 8 tiles of 128 tokens

_CACHE = {}


def _build():
    nc = bacc.Bacc("TRN2", num_devices=NCORES)
    A3p = nc.dram_tensor("A3p", [M1 * 4, 128], dt.bfloat16, kind="ExternalInput")
    B3p = nc.dram_tensor("B3p", [M2 * 4, 128], dt.bfloat16, kind="ExternalInput")
    idxA = nc.dram_tensor("idxA", [128, 256], dt.int16, kind="ExternalInput")
    idxB = nc.dram_tensor("idxB", [128, 256], dt.int16, kind="ExternalInput")
    # out[t, a, cb, h, (k'', b)]
    out = nc.dram_tensor("out", [NTILES, 64, 4, 2, 512], dt.bfloat16,
                         kind="ExternalOutput")

    with tile.TileContext(nc) as tc, ExitStack() as ctx:
        const_pool = ctx.enter_context(tc.tile_pool(name="const", bufs=1))
        ev_pool = ctx.enter_context(tc.tile_pool(name="ev", bufs=3))
        ps_pool = ctx.enter_context(tc.tile_pool(name="ps", bufs=6, space="PSUM"))

        idxA_sb = const_pool.tile([128, 256], dt.int16, tag="idxA")
        idxB_sb = const_pool.tile([128, 256], dt.int16, tag="idxB")
        nc.sync.dma_start(idxA_sb[:], idxA[:])
        nc.sync.dma_start(idxB_sb[:], idxB[:])

        Agp = const_pool.tile([128, 32, 128], dt.bfloat16, tag="Agp")
        Bgp = const_pool.tile([128, 32, 128], dt.bfloat16, tag="Bgp")

        # Block-diagonal moving operand for a 4-tile chunk:
        #   bd[4k'+p, t4, cb, sigma, h, 32*(k'%16)+b] = B[2p+sigma, j_tok, b]
        # (h = k'//16 selects the psum half). Zeros memset once; diagonal
        # cells are rewritten by every chunk's scatter.
        bd = const_pool.tile([128, 4, 4, 2, 2, 512], dt.bfloat16, tag="bd")
        nc.vector.memset(bd[:], 0.0)

        for t in range(NTILES):
            nc.gpsimd.dma_gather(
                Agp[:, 4 * t:4 * t + 4, :], A3p[:],
                idxA_sb[:, 32 * t:32 * t + 32], 512, 512, 128)
            nc.gpsimd.dma_gather(
                Bgp[:, 4 * t:4 * t + 4, :], B3p[:],
                idxB_sb[:, 32 * t:32 * t + 32], 512, 512, 128)

        BD_ROW = 4 * 4 * 2 * 2 * 512           # 32768 els per partition
        BG_ROW = 32 * 128                      # 4096 els per partition
        for chunk in range(2):
            # scatter: one DMA per k' slot, covering (t4, cb, sigma, b)
            for kp in range(32):
                h, kk = kp // 16, kp % 16
                eng = nc.sync if kp % 2 == 0 else nc.scalar
                dst = bass.AP(
                    bd[:].tensor,
                    bd[:].offset + (4 * kp) * BD_ROW + h * 512 + 32 * kk,
                    [[BD_ROW, 4], [1024, 32], [1, 32]],
                )
                src = bass.AP(
                    Bgp[:].tensor,
                    Bgp[:].offset + (4 * kp) * BG_ROW + chunk * 16 * 128,
                    [[BG_ROW, 4], [64, 32], [1, 32]],
                )
                eng.dma_start(dst, src)

            for t4 in range(4):
                t = 4 * chunk + t4
                ev = ev_pool.tile([64, 4, 2, 512], dt.bfloat16, tag="ev")
                for cb in range(4):
                    c = 4 * t + cb
                    for h in range(2):
                        ps = ps_pool.tile([64, 512], dt.float32, tag="ps")
                        for sg in range(2):
                            nc.tensor.matmul(
                                ps[:],
                                Agp[:, c, 64 * sg:64 * sg + 64],
                                bd[:, t4, cb, sg, h, :],
                                start=(sg == 0),
                                stop=(sg == 1),
                            )
                        if (2 * cb + h) % 2 == 0:
                            nc.vector.tensor_copy(ev[:, cb, h, :], ps[:])
                        else:
                            nc.scalar.copy(ev[:, cb, h, :], ps[:])
                nc.scalar.dma_start(out[t], ev[:])

    nc.compile()
    return nc


def _wrap_idxs(idx: np.ndarray) -> np.ndarray:
    """[n] -> SWDGE wrapped layout [128, n//16] int16 (16-wrap, 8x replicated)."""
    n = idx.shape[0]
    w = idx.reshape(n // 16, 16).T.astype(np.int16)
    return np.ascontiguousarray(np.tile(w, (8, 1)))


def _prep(A, B, x):
    A = np.asarray(A, dtype=np.float32)
    B = np.asarray(B, dtype=np.float32)
    xl = np.asarray(x).astype(np.int64).reshape(-1)           # [8192]

    Ab = A.astype(ml_dtypes.bfloat16)                          # [8, 224, 64]
    Bb = B.astype(ml_dtypes.bfloat16)                          # [8, 224, 32]
    A3p = np.zeros((M1 * 4, 128), dtype=ml_dtypes.bfloat16)
    B3p = np.zeros((M2 * 4, 128), dtype=ml_dtypes.bfloat16)
    for p in range(4):
        A3p[p::4, 0:64] = Ab[2 * p]
        A3p[p::4, 64:128] = Ab[2 * p + 1]
        B3p[p::4, 0:32] = Bb[2 * p]
        B3p[p::4, 64:96] = Bb[2 * p + 1]

    i_all = (xl // M2).astype(np.int64)
    j_all = (xl % M2).astype(np.int64)

    p4 = np.arange(4, dtype=np.int64)
    in_maps = []
    for c in range(NCORES):
        sl = slice(c * TPC, (c + 1) * TPC)
        ia = (i_all[sl, None] * 4 + p4[None, :]).reshape(-1)   # [4096]
        jb = (j_all[sl, None] * 4 + p4[None, :]).reshape(-1)
        in_maps.append(
            dict(A3p=A3p, B3p=B3p, idxA=_wrap_idxs(ia), idxB=_wrap_idxs(jb))
        )
    return in_maps


def _unpack(o: np.ndarray) -> np.ndarray:
    """Device out [8, 64, 4, 2, 512] -> [1024, 2048] token-major fp32.

    out[t, a, cb, h, k''*32+b] is token (4t+cb)*32 + 16h + k''."""
    o = np.asarray(o).astype(np.float32)
    o = o.reshape(NTILES, 64, 4, 2, 16, 32)        # [t, a, cb, h, k2, b]
    o = o.transpose(0, 2, 3, 4, 1, 5)              # [t, cb, h, k2, a, b]
    return np.ascontiguousarray(o.reshape(TPC, EMB))


def kernel(A: np.ndarray, B: np.ndarray, x: np.ndarray) -> np.ndarray:
    in_maps = _prep(A, B, x)
    if "nc" not in _CACHE:
        _CACHE["nc"] = _build()
    nc = _CACHE["nc"]
    _CACHE["in_maps"] = in_maps

    res = bass_utils.run_bass_kernel_spmd(nc, in_maps, core_ids=list(range(NCORES)))

    outs = [_unpack(res.results[c]["out"]) for c in range(NCORES)]
    full = np.concatenate(outs, axis=0)                # [8192, 2048]
    return full.reshape(BATCH, SEQ, EMB)
